# revision 6
# baseline (speedup 1.0000x reference)
"""LAGCNII full-device kernel for 8 TRN2 NeuronCores.

Node-sharded (12544 padded rows/core). Per layer l (W'_l=(1-b)I+b*W_l):
  u = h @ (0.9 W'_l)   [node-major, per 128-row window, PE]
  c = h0 @ (0.1 W'_l)  [SBUF-resident]
  AllGather(u) -> replicated u across cores (bf16)
  router: SWDGE indirect-gather of incoming source rows in dst-sorted
          order (302 tiles of 128 rows/layer), PE one-hot reduce with
          PSUM accumulation -> agg windows
  h_next = relu(agg + c)
Destination windows are tiered + in-degree balanced (LPT): per core,
90 light windows (<=384 slots, 3 router tiles) and 8 heavy windows of
exactly-degree-4 nodes (512 slots, 4 tiles) -> 302 router tiles/layer,
within 2% of the ceil(E/8/128)=296 floor. The AllGather is split 64/34
windows so the first chunk can fire before the dense pass finishes.
Output fp32.
"""
import numpy as np
import ml_dtypes

BF16 = ml_dtypes.bfloat16

N = 100000
E = 300000
PER = 12500
NL = 12544           # 98*128
W = 98
CH = 256
DH = 128
C = 40
L = 8
NC = 8
THETA = 0.5

WL = 90                   # light windows (<=384 slots, 3 tiles)
WH = 8                    # heavy windows (deg-4 nodes, <=512 slots, 4 tiles)
RT = WL * 3 + WH * 4      # 302 router tiles
TOFF = [3 * w for w in range(WL)] + [WL * 3 + 4 * k for k in range(WH)]
SCHED = []                # (window, start, stop) per router tile
for _w in range(W):
    _nt = 3 if _w < WL else 4
    for _k in range(_nt):
        SCHED.append((_w, _k == 0, _k == _nt - 1))
WA = 64                   # windows in first AllGather chunk
RA = WA * 128             # 8192 rows
RB = NL - RA              # 4352 rows

_cached = {}


def _lpt(deg, nwin):
    """LPT-pack len(deg) items into nwin windows of <=128 items each;
    returns (window, pos) per item and window loads."""
    import heapq
    order = np.argsort(-deg, kind="stable")
    heap = [(0, w) for w in range(nwin)]
    heapq.heapify(heap)
    counts = np.zeros(nwin, np.int32)
    loads = np.zeros(nwin, np.int64)
    win = np.empty(len(deg), np.int64)
    pos = np.empty(len(deg), np.int64)
    for n in order:
        while True:
            load, w = heapq.heappop(heap)
            if counts[w] < 128:
                break
        win[n] = w
        pos[n] = counts[w]
        counts[w] += 1
        loads[w] = load + int(deg[n])
        heapq.heappush(heap, (loads[w], w))
    return win, pos, loads


def _pack_windows(deg):
    """Tiered packing: 8 heavy windows of 128 deg<=4 nodes (<=512 slots),
    90 light windows (<=384 slots). Returns local slot per node."""
    idx4 = np.where(deg == 4)[0]
    if len(idx4) >= WH * 128:
        heavy = idx4[:WH * 128]
    else:
        idx3 = np.where(deg == 3)[0]
        heavy = np.concatenate([idx4, idx3[:WH * 128 - len(idx4)]])
    if len(heavy) < WH * 128:
        raise RuntimeError("not enough deg-3/4 nodes for heavy windows")
    hm = np.zeros(PER, bool)
    hm[heavy] = True
    lights = np.where(~hm)[0]
    if deg[lights].sum() > WL * 384:
        raise RuntimeError("light slot budget exceeded")

    slot = np.empty(PER, np.int64)
    # heavy: any 128-grouping works (deg<=4 -> <=512 slots)
    for k in range(WH):
        grp = heavy[k * 128:(k + 1) * 128]
        slot[grp] = (WL + k) * 128 + np.arange(128)
    lwin, lpos, loads = _lpt(deg[lights], WL)
    if loads.max() > 384:
        raise RuntimeError(f"light window overflow {loads.max()}")
    slot[lights] = lwin * 128 + lpos
    return slot


def _prep(edge_index):
    src = edge_index[0].astype(np.int64)
    dst = edge_index[1].astype(np.int64)
    deg_in = np.bincount(dst, minlength=N)

    newpos = np.empty(N, np.int64)
    for c in range(NC):
        g0 = c * PER
        slots = _pack_windows(deg_in[g0:g0 + PER])
        newpos[g0:g0 + PER] = c * NL + slots

    s = newpos[src]
    d = newpos[dst]
    d_core, d_loc = d // NL, d % NL

    order = np.argsort(d_core * NL + d_loc, kind="stable")
    dr_core = d_core[order]
    dr_loc = d_loc[order]
    sr_row = s[order]
    dwin = dr_loc // 128
    key_win = dr_core * W + dwin          # nondecreasing
    cnt_win = np.bincount(key_win, minlength=NC * W)
    cap = np.where(np.arange(NC * W) % W < WL, 384, 512)
    if (cnt_win > cap).any():
        raise RuntimeError("recv window overflow")
    win_start = np.searchsorted(key_win, np.arange(NC * W))
    qpos = np.arange(E) - win_start[key_win]

    gidx = np.zeros((NC, 128, RT), dtype=np.int32)
    oh_r = np.zeros((NC, RT, 128, 128), dtype=np.float32)
    t_idx = np.asarray(TOFF)[dwin] + qpos // 128
    p_idx = qpos % 128
    # split-AllGather layout: [8 x RA | 8 x RB]
    s_core, s_loc = sr_row // NL, sr_row % NL
    row = np.where(s_loc < RA, s_core * RA + s_loc,
                   NC * RA + s_core * RB + (s_loc - RA))
    gidx[dr_core, p_idx, t_idx] = row
    oh_r[dr_core, t_idx, p_idx, dr_loc % 128] = 1.0

    return {"newpos": newpos, "oh_r": oh_r.astype(BF16), "gidx": gidx}


def _build():
    if "nc" in _cached:
        return _cached["nc"]
    import concourse.bacc as bacc
    import concourse.mybir as mybir
    import concourse.tile as tile
    import concourse.bass as bass

    bf = mybir.dt.bfloat16
    f32 = mybir.dt.float32

    nc = bacc.Bacc("TRN2", target_bir_lowering=False, debug=False,
                   num_devices=NC)
    x0T = nc.dram_tensor("x0T", [CH, NL], bf, kind="ExternalInput")
    x1T = nc.dram_tensor("x1T", [CH, NL], bf, kind="ExternalInput")
    lw = nc.dram_tensor("lw", [2, CH, DH], bf, kind="ExternalInput")
    lbT = nc.dram_tensor("lbT", [DH, 2], f32, kind="ExternalInput")
    wu = nc.dram_tensor("wu", [L, CH, CH], bf, kind="ExternalInput")
    wc = nc.dram_tensor("wc", [L, CH, CH], bf, kind="ExternalInput")
    ow = nc.dram_tensor("ow", [CH, C], bf, kind="ExternalInput")
    ohr_d = nc.dram_tensor("ohr", [RT, 128, 128], bf, kind="ExternalInput")
    gidx_d = nc.dram_tensor("gidx", [128, RT], mybir.dt.int32,
                            kind="ExternalInput")
    out_d = nc.dram_tensor("out", [NL, C], f32, kind="ExternalOutput")

    h_d = [nc.dram_tensor(f"h{i}", [NL, CH], bf, kind="Internal")
           for i in range(2)]
    u_d = nc.dram_tensor("u", [NL, CH], bf, kind="Internal")
    uag = [nc.dram_tensor(f"uag{i}", [NC * NL, CH], bf, kind="Internal",
                          addr_space="Shared") for i in range(2)]

    relu = mybir.ActivationFunctionType.Relu

    with tile.TileContext(nc) as tc:
        with (
            tc.tile_pool(name="cst", bufs=1) as cst,
            tc.tile_pool(name="big", bufs=1) as big,
            tc.tile_pool(name="wp", bufs=2) as wp,
            tc.tile_pool(name="hp", bufs=3) as hp,
            tc.tile_pool(name="gp", bufs=8) as gp,
            tc.tile_pool(name="sp", bufs=4) as sp,
            tc.tile_pool(name="ps", bufs=3, space="PSUM") as ps,
            tc.tile_pool(name="psa", bufs=2, space="PSUM") as psa,
        ):
            h0T_a = big.tile([128, NL], bf, tag="h0Ta")
            h0T_b = big.tile([128, NL], bf, tag="h0Tb")
            c_all = big.tile([128, W * CH], bf, tag="c")
            gidx_t = cst.tile([128, RT], mybir.dt.int32, tag="gidx")
            nc.sync.dma_start(out=gidx_t[:], in_=gidx_d[:])
            lbT_t = cst.tile([DH, 2], f32, tag="lbT")
            nc.sync.dma_start(out=lbT_t[:], in_=lbT[:])

            # setup: h0T = relu(lw.T @ xT + b)  (feat-major halves)
            for view, (xT, h0T) in enumerate(((x0T, h0T_a), (x1T, h0T_b))):
                lwt = wp.tile([128, 2, DH], bf, tag="lwt")
                nc.sync.dma_start(
                    out=lwt[:],
                    in_=lw[view].rearrange("(k p) d -> p k d", p=128))
                for ch0 in range(0, NL, 512):
                    cw = min(512, NL - ch0)
                    xt = hp.tile([128, 2, 512], bf, tag="xt")
                    nc.sync.dma_start(
                        out=xt[:, :, :cw],
                        in_=xT[:, ch0:ch0 + cw].rearrange(
                            "(k p) n -> p k n", p=128))
                    p0 = ps.tile([DH, 512], f32, tag="mm")
                    for k in range(2):
                        nc.tensor.matmul(out=p0[:, :cw], lhsT=lwt[:, k, :],
                                         rhs=xt[:, k, :cw],
                                         start=(k == 0), stop=(k == 1))
                    nc.scalar.activation(
                        out=h0T[:, ch0:ch0 + cw], in_=p0[:, :cw], func=relu,
                        bias=lbT_t[:, view:view + 1])

            for l in range(L):
                wu_t = wp.tile([128, 2, CH], bf, tag="wu")
                wc_t = wp.tile([128, 2, CH], bf, tag="wc")
                nc.sync.dma_start(
                    out=wu_t[:], in_=wu[l].rearrange("(k p) d -> p k d", p=128))
                nc.sync.dma_start(
                    out=wc_t[:], in_=wc[l].rearrange("(k p) d -> p k d", p=128))

                # dense u, c per window (node-major)
                for ch0 in range(0, NL, 512):
                    cw = min(512, NL - ch0)
                    if l == 0:
                        ta, tb, off = h0T_a, h0T_b, ch0
                    else:
                        ta = hp.tile([128, 512], bf, tag="hta")
                        tb = hp.tile([128, 512], bf, tag="htb")
                        nc.sync.dma_start_transpose(
                            out=ta[:, :cw],
                            in_=h_d[(l + 1) % 2][ch0:ch0 + cw, 0:128])
                        nc.sync.dma_start_transpose(
                            out=tb[:, :cw],
                            in_=h_d[(l + 1) % 2][ch0:ch0 + cw, 128:256])
                        off = 0
                    for wo in range(cw // 128):
                        w = (ch0 + wo * 128) // 128
                        sl = slice(off + wo * 128, off + (wo + 1) * 128)
                        pu = ps.tile([128, CH], f32, tag="mm")
                        nc.tensor.matmul(out=pu[:], lhsT=ta[:, sl],
                                         rhs=wu_t[:, 0, :],
                                         start=True, stop=False)
                        nc.tensor.matmul(out=pu[:], lhsT=tb[:, sl],
                                         rhs=wu_t[:, 1, :],
                                         start=False, stop=True)
                        ut = sp.tile([128, CH], bf, tag="ut")
                        nc.vector.tensor_copy(out=ut[:], in_=pu[:])
                        nc.sync.dma_start(
                            out=u_d[w * 128:(w + 1) * 128, :], in_=ut[:])
                        pc = ps.tile([128, CH], f32, tag="mm")
                        nc.tensor.matmul(
                            out=pc[:], lhsT=h0T_a[:, w * 128:(w + 1) * 128],
                            rhs=wc_t[:, 0, :], start=True, stop=False)
                        nc.tensor.matmul(
                            out=pc[:], lhsT=h0T_b[:, w * 128:(w + 1) * 128],
                            rhs=wc_t[:, 1, :], start=False, stop=True)
                        nc.vector.tensor_copy(
                            out=c_all[:, w * CH:(w + 1) * CH], in_=pc[:])

                # exchange: replicate u (split so chunk A can fire early)
                uag_t = uag[l % 2]
                nc.gpsimd.collective_compute(
                    "AllGather", mybir.AluOpType.bypass,
                    replica_groups=[list(range(NC))],
                    ins=[u_d[0:RA, :]], outs=[uag_t[0:NC * RA, :]],
                )
                nc.gpsimd.collective_compute(
                    "AllGather", mybir.AluOpType.bypass,
                    replica_groups=[list(range(NC))],
                    ins=[u_d[RA:NL, :]], outs=[uag_t[NC * RA:NC * NL, :]],
                )

                # router + reduce
                for t in range(RT):
                    w, t_start, t_stop = SCHED[t]
                    gat = gp.tile([128, CH], bf, tag="gat")
                    nc.gpsimd.indirect_dma_start(
                        out=gat[:], out_offset=None, in_=uag_t[:],
                        in_offset=bass.IndirectOffsetOnAxis(
                            ap=gidx_t[:, t:t + 1], axis=0))
                    ohrt = gp.tile([128, 128], bf, tag="ohr")
                    nc.sync.dma_start(out=ohrt[:], in_=ohr_d[t])
                    if t_start:
                        pa = psa.tile([128, CH], f32, tag="pa")
                    nc.tensor.matmul(out=pa[:], lhsT=ohrt[:], rhs=gat[:],
                                     start=t_start, stop=t_stop)
                    if t_stop:
                        hn = sp.tile([128, CH], f32, tag="hn")
                        nc.vector.tensor_tensor(
                            out=hn[:], in0=pa[:],
                            in1=c_all[:, w * CH:(w + 1) * CH],
                            op=mybir.AluOpType.add)
                        ho = sp.tile([128, CH], bf, tag="ho")
                        nc.vector.tensor_scalar(
                            out=ho[:], in0=hn[:], scalar1=0.0, scalar2=None,
                            op0=mybir.AluOpType.max)
                        nc.sync.dma_start(
                            out=h_d[l % 2][w * 128:(w + 1) * 128, :], in_=ho[:])

            # final projection
            owt = wp.tile([128, 2, C], bf, tag="ow")
            nc.sync.dma_start(
                out=owt[:], in_=ow.rearrange("(k p) d -> p k d", p=128))
            for ch0 in range(0, NL, 512):
                cw = min(512, NL - ch0)
                ta = hp.tile([128, 512], bf, tag="fta")
                tb = hp.tile([128, 512], bf, tag="ftb")
                nc.sync.dma_start_transpose(
                    out=ta[:, :cw], in_=h_d[(L - 1) % 2][ch0:ch0 + cw, 0:128])
                nc.sync.dma_start_transpose(
                    out=tb[:, :cw], in_=h_d[(L - 1) % 2][ch0:ch0 + cw, 128:256])
                for wo in range(cw // 128):
                    po = ps.tile([128, C], f32, tag="mm")
                    nc.tensor.matmul(
                        out=po[:], lhsT=ta[:, wo * 128:(wo + 1) * 128],
                        rhs=owt[:, 0, :], start=True, stop=False)
                    nc.tensor.matmul(
                        out=po[:], lhsT=tb[:, wo * 128:(wo + 1) * 128],
                        rhs=owt[:, 1, :], start=False, stop=True)
                    ot = sp.tile([128, C], f32, tag="ot")
                    nc.vector.tensor_copy(out=ot[:], in_=po[:])
                    nc.sync.dma_start(
                        out=out_d[ch0 + wo * 128:ch0 + (wo + 1) * 128, :],
                        in_=ot[:])
    nc.compile()
    _cached["nc"] = nc
    return nc


def _in_maps(x0, x1, edge_index, lin_w, lin_b, gcn_w, out_w):
    ei = np.asarray(edge_index)
    key = ei.tobytes()[:256]
    if _cached.get("prep_key") != key:
        _cached["prep"] = _prep(ei)
        _cached["prep_key"] = key
    prep = _cached["prep"]
    newpos = prep["newpos"]

    betas = np.log(THETA / np.arange(1, L + 1, dtype=np.float32) + 1.0)
    eye = np.eye(CH, dtype=np.float32)
    wu = np.stack([0.9 * ((1 - b) * eye + b * np.asarray(gcn_w[i], np.float32))
                   for i, b in enumerate(betas)]).astype(BF16)
    wc = np.stack([0.1 * ((1 - b) * eye + b * np.asarray(gcn_w[i], np.float32))
                   for i, b in enumerate(betas)]).astype(BF16)

    x0p = np.zeros((NC * NL, CH), np.float32)
    x1p = np.zeros((NC * NL, CH), np.float32)
    x0p[newpos] = np.asarray(x0, np.float32)
    x1p[newpos] = np.asarray(x1, np.float32)
    x0p = x0p.astype(BF16)
    x1p = x1p.astype(BF16)

    maps = []
    for c in range(NC):
        maps.append({
            "x0T": np.ascontiguousarray(x0p[c * NL:(c + 1) * NL].T),
            "x1T": np.ascontiguousarray(x1p[c * NL:(c + 1) * NL].T),
            "lw": np.asarray(lin_w, np.float32).astype(BF16),
            "lbT": np.ascontiguousarray(np.asarray(lin_b, np.float32).T),
            "wu": wu,
            "wc": wc,
            "ow": np.asarray(out_w, np.float32).astype(BF16),
            "ohr": prep["oh_r"][c],
            "gidx": prep["gidx"][c],
        })
    return maps, newpos


def kernel(x0, x1, edge_index, lin_w, lin_b, gcn_w, out_w, out_b):
    from concourse import bass_utils

    maps, newpos = _in_maps(x0, x1, edge_index, lin_w, lin_b, gcn_w, out_w)
    nc = _build()
    res = bass_utils.run_bass_kernel_spmd(
        nc, maps, core_ids=list(range(NC)), trace=False)
    dev = np.concatenate([res.results[c]["out"] for c in range(NC)], axis=0)
    out = dev[newpos].astype(np.float32)
    out += np.asarray(out_b, np.float32)[None, :]
    return out


def _install_ntff_shim():
    """Register the axon NTFF profile hook if the image's antenv lacks it."""
    import contextlib
    import ctypes
    import sys
    import types
    try:
        import antenv.axon_hooks  # noqa: F401
        return
    except ImportError:
        pass
    try:
        lib = ctypes.CDLL("/opt/axon/libaxon_pjrt.so")
    except OSError:
        return
    if not hasattr(lib, "axon_start_nrt_profile"):
        return
    lib.axon_start_nrt_profile.argtypes = [ctypes.POINTER(ctypes.c_int64),
                                           ctypes.c_size_t]
    lib.axon_start_nrt_profile.restype = ctypes.c_int64
    lib.axon_stop_nrt_profile.argtypes = [ctypes.c_char_p]
    lib.axon_stop_nrt_profile.restype = ctypes.c_int64

    @contextlib.contextmanager
    def _hook(output_dir, device_ids):
        import jax
        jax.devices()
        if device_ids:
            ids = (ctypes.c_int64 * len(device_ids))(*device_ids)
            rc = lib.axon_start_nrt_profile(ids, len(device_ids))
        else:
            rc = lib.axon_start_nrt_profile(None, 0)
        if rc != 0:
            raise RuntimeError(f"axon_start_nrt_profile rc={rc}")
        try:
            yield
        finally:
            lib.axon_stop_nrt_profile(str(output_dir).encode())

    mod = types.ModuleType("antenv.axon_hooks")
    state = {"hook": _hook}
    mod.get_axon_ntff_profile_hook = lambda: state["hook"]
    mod.set_axon_ntff_profile_hook = lambda h: state.update(hook=h)
    sys.modules["antenv.axon_hooks"] = mod
    try:
        import antenv
        antenv.axon_hooks = mod
    except ImportError:
        pass


def profile(inputs):
    """Run once with NTFF tracing, return exec_time_ns (for test.py)."""
    from concourse import bass_utils

    _install_ntff_shim()

    maps, _ = _in_maps(inputs["x0"], inputs["x1"], inputs["edge_index"],
                       inputs["lin_w"], inputs["lin_b"], inputs["gcn_w"],
                       inputs["out_w"])
    nc = _build()
    res = bass_utils.run_bass_kernel_spmd(
        nc, maps, core_ids=list(range(NC)), trace=True)
    return res.exec_time_ns
